# revision 28
# baseline (speedup 1.0000x reference)
"""Trainium2 Bass kernel for nn_ODEFunc_interaction (gnn_message_passing).

Math (see reference):
  dz_dt = tanh([z, t] @ vW1 + vb1) @ vW2 + vb2                    (v-net, all rows)
  for each pair (perm[2i], perm[2i+1]):
      d_i  = z[perm[2i]] - z[perm[2i+1]]
      v    = tanh(d_i @ pW1 + pb1);  q = (pW1*pW2)^T v^2;  c0 = sum(pW1*pW2)
      out[perm[2i]]   = dz_dt[perm[2i]]   + q - c0   (+vb2)
      out[perm[2i+1]] = dz_dt[perm[2i+1]] - q + c0   (+vb2)
  last 3 rows (triple) handled on host in float64 (tiny).

Mapping: 8 cores, data-parallel over pairs. Per core 25000 rows = 12500 pairs,
packed 4 chunks deep in the partition dim: partition 32j+d = dim d of chunk j.
Column space (per chunk) = 3136 padded pairs in superblocks of 512 pairs
(tail 64), each superblock ordered [512 even rows | 512 odd rows].

Engine mapping:
 - PE, addressed as 32-wide tiles (tile_position): h/pa matmuls (K=32) as
   concurrent row-tiles (32j,0); dz/q (M=32) as concurrent col-tiles (0,32j)
   with shared stationaries. Each in-flight matmul owns a full PSUM bank
   (concurrent drains into one bank wedge the device). A warm-up burst of
   dummy matmuls during the initial DMA window brings the PE out of the
   HAM half-clock state before real work starts.
 - ACT computes only the h tanh (split in chunk-halves A/B so the psum
   ping-pong never stalls the ACT queue).
 - DVE computes tanh^2 for the pair branch in ONE custom op per chunk
   (deg-5 odd polynomial + clamp: min((y(c1+t(c2+t*c3)))^2, C), t=y^2,
   max err 0.022 -> q err ~0.014 abs, well within tolerance; pb1 is zero
   for this problem which frees the bias const slot). DVE also evacuates
   dz psum -> fp16 SBUF.
 - q accumulates into the dz psum region with +/- weights, so the final
   even/odd combine is free.
 - GPSIMD computes the pair diffs.
"""

import os
import numpy as np

B, D, H = 200003, 32, 128
NCORES = 8
P2 = 200000              # rows covered by pairs
RPC = P2 // NCORES       # 25000 rows per core
NCHUNK = 4
ROWS_PC = RPC // NCHUNK  # 6250 rows per chunk
PAIRS_PC = ROWS_PC // 2  # 3125 pairs per chunk
HALFP = 3136             # padded pairs per chunk: 64 + 6*512
SBW = 512                # pairs per superblock
TAILW = HALFP - 6 * SBW  # 64; placed FIRST so the pipeline starts on a
                         # small cheap transfer
SBS = [TAILW] + [SBW] * 6
# pair range (start, count) per superblock: tail covers the last (padded) pairs
SB_PAIRS = [(6 * SBW, TAILW)] + [(i * SBW, SBW) for i in range(6)]
XC = 2 * HALFP           # 6272 columns per core

# deg-5-odd tanh^2 approx: min((y*(c1 + t*(c2 + t*c3)))^2, CLAMP), t = y*y
TSQ_C1, TSQ_C2, TSQ_C3 = 0.91987675, -0.17231731, 0.0153519
TSQ_CLAMP = 0.97771559

_CACHE = {}
LAST_RESULTS = None      # BassKernelResults of the most recent run (for test.py)


def _tanhsq_ref(in0, in1, s0, s1, imm2):
    x = in0.astype(np.float32)
    t = x * x
    v = x * (s0 + t * (s1 + t * imm2))
    return np.minimum(v * v, in1).astype(np.float32)


def _register_tanhsq():
    """Register the TANH_SQ_ANT custom-DVE op (8 uop stages)."""
    if "op" in _CACHE.setdefault("tanhsq", {}):
        return _CACHE["tanhsq"]["op"]
    from concourse import dve_ops
    from concourse.dve_spec import (
        Spec, Src0, C0, C1, C2, C3, minn, sq, _spill_c3_to_src1, lower, _has_src1,
    )
    from concourse.dve_uop import DveOpSpec

    name = "TANH_SQ_ANT"
    if name not in dve_ops._SUB_OPCODE_FOR_NAME:
        t = Src0 * Src0
        v = Src0 * (C0 + t * (C1 + t * C2))
        body = _spill_c3_to_src1(minn(sq(v), C3))
        spec = Spec(body=body, reference=_tanhsq_ref)
        row = 1 + len(dve_ops.OPS)
        assert row < 0x20
        dve_ops._SUB_OPCODE_FOR_NAME[name] = row
        shas = {}
        for ver in ("v3", "v4"):
            uops = lower(spec, ver=ver)
            shas[ver] = DveOpSpec(
                name=name, opcode=row, uops=uops, rd1_en=_has_src1(spec)
            ).sha(ver)
        op = dve_ops.DveOp(name, spec, subdim=False, uops_sha=shas)
        dve_ops.OPS.append(op)
        dve_ops.CUSTOM_DVE_SPECS[name] = spec
    else:
        op = next(o for o in dve_ops.OPS if o.name == name)
    _CACHE["tanhsq"]["op"] = op
    return op


def build_program(use_tanhsq=True):
    from contextlib import ExitStack
    import concourse.bacc as bacc
    import concourse.mybir as mybir
    import concourse.tile as tile

    tanhsq_op = _register_tanhsq() if use_tanhsq else None

    dt = mybir.dt
    F16, F32 = dt.float16, dt.float32
    AF = mybir.ActivationFunctionType
    OP = mybir.AluOpType

    nc = bacc.Bacc()
    X = nc.dram_tensor("x", [128, XC], F16, kind="ExternalInput")
    WT = nc.dram_tensor("wcat", [128, 352], F16, kind="ExternalInput")
    BT = nc.dram_tensor("bias", [128, 3], F32, kind="ExternalInput")
    O = nc.dram_tensor("out", [128, XC], F16, kind="ExternalOutput")

    with tile.TileContext(nc) as tc, ExitStack() as ctx:
        wpool = ctx.enter_context(tc.tile_pool(name="wpool", bufs=1))
        xpool = ctx.enter_context(tc.tile_pool(name="xpool", bufs=2))
        dfpool = ctx.enter_context(tc.tile_pool(name="dfpool", bufs=2))
        upool = ctx.enter_context(tc.tile_pool(name="upool", bufs=6))
        sqpool = ctx.enter_context(tc.tile_pool(name="sqpool", bufs=2))
        vpool = None
        if not use_tanhsq:
            vpool = ctx.enter_context(tc.tile_pool(name="vpool", bufs=2))
        opool = ctx.enter_context(tc.tile_pool(name="opool", bufs=2))
        # PSUM: ph 2x2 banks + pa 1x2 banks + dz 1x2 banks = 8 exactly
        hps = ctx.enter_context(tc.tile_pool(name="hps", bufs=2, space="PSUM"))
        papool = ctx.enter_context(tc.tile_pool(name="papool", bufs=1, space="PSUM"))
        dzpool = ctx.enter_context(tc.tile_pool(name="dzpool", bufs=1, space="PSUM"))

        # PE warm-up: ~4us of dummy matmuls on a zeroed tile, overlapping the
        # initial input DMA. Brings HAM out of the half-clock state so real
        # matmuls run closer to 2.4 GHz. A dummy activation forces the Tanh
        # table load before the first real data arrives.
        zt = wpool.tile([128, 644], F16)
        nc.gpsimd.memset(zt[:], 0.0)
        nc.scalar.activation(zt[:, 640:641], zt[:, 0:1], AF.Tanh)
        warm = hps.tile([128, 1024], F32, tag="ph", name="warm")
        for _ in range(6):
            nc.tensor.matmul(warm[:, 0:512], zt[:, 0:128], zt[:, 128:640],
                             start=True, stop=True)

        # first input superblock (the small tail) before the weight tensors:
        # the pipeline starts on a cheap transfer
        W0 = SBS[0]
        xt0 = xpool.tile([128, 2 * SBW], F16, tag="xt", name="xt0")
        nc.sync.dma_start(xt0[:, : 2 * W0], X[:, 0 : 2 * W0])

        wt = wpool.tile([128, 352], F16)
        nc.sync.dma_start(wt[:], WT[:])
        bt = wpool.tile([128, 3], F32)
        nc.sync.dma_start(bt[:], BT[:])
        w1 = wt[:, 0:128]      # [32j+d, h] = vW1[d, h]
        pw1 = wt[:, 128:256]   # [32j+d, h] = pW1[d, h]
        w2 = wt[:, 256:288]    # [h, d] = vW2[h, d]
        pwp = wt[:, 288:320]   # [h, d] = pW1[d, h] * pW2[h]
        pwn = wt[:, 320:352]   # -pwp
        bh = bt[:, 0:1]        # vb1 + t * vW1[32]
        pb1 = bt[:, 1:2]
        clamp = bt[:, 2:3]

        def h_half(xt, w, V, half, k):
            """h pre-acts for half-SB w (even or odd rows), chunks (2*half,
            2*half+1): 2 concurrent row-tiles, chunk c -> own psum bank;
            then one tanh -> fp16. V = cols per chunk strip."""
            ph = hps.tile([128, 1024], F32, tag="ph", name=f"ph{k}_{w}_{half}")
            for c in range(2):
                j = 2 * half + c
                p0 = 32 * j
                nc.tensor.matmul(
                    ph[:, 512 * c : 512 * c + V],
                    w1[p0 : p0 + 32, :],
                    xt[p0 : p0 + 32, 512 * w : 512 * w + V],
                    start=True, stop=True,
                    tile_position=(p0, 0),
                )
            ut = upool.tile([128, 1024], F16, tag="ut", name=f"ut{k}_{w}_{half}")
            if V == 512:
                nc.scalar.activation(ut[:, :], ph[:, :], AF.Tanh, bias=bh[:])
            else:
                for c in range(2):
                    nc.scalar.activation(
                        ut[:, 512 * c : 512 * c + V],
                        ph[:, 512 * c : 512 * c + V],
                        AF.Tanh, bias=bh[:],
                    )
            return ut

        def pa_half(half, dft, sq, W, k):
            """pa pre-acts for chunks (2*half, 2*half+1) -> [128,1024], chunk
            c at bank 512c, then one fused tanh^2 into the shared sq tile via
            the custom DVE op (or ACT fallback)."""
            pap = papool.tile([128, 1024], F32, tag="pap", name=f"pap{k}_{half}")
            for c in range(2):
                j = 2 * half + c
                p0 = 32 * j
                nc.tensor.matmul(
                    pap[:, 512 * c : 512 * c + W],
                    pw1[p0 : p0 + 32, :],
                    dft[p0 : p0 + 32, :W],
                    start=True, stop=True,
                    tile_position=(p0, 0),
                )
            if use_tanhsq:
                if W == SBW:
                    nc.vector._custom_dve(
                        tanhsq_op,
                        out=sq[:, 1024 * half : 1024 * half + 1024],
                        in0=pap[:, :],
                        in1=clamp,
                        s0=TSQ_C1, s1=TSQ_C2, imm2=TSQ_C3,
                    )
                else:
                    for c in range(2):
                        j = 2 * half + c
                        nc.vector._custom_dve(
                            tanhsq_op,
                            out=sq[:, 512 * j : 512 * j + W],
                            in0=pap[:, 512 * c : 512 * c + W],
                            in1=clamp,
                            s0=TSQ_C1, s1=TSQ_C2, imm2=TSQ_C3,
                        )
            else:
                for c in range(2):
                    j = 2 * half + c
                    vt = vpool.tile([128, 512], F16, tag="vt", name=f"vt{k}_{j}")
                    nc.scalar.activation(vt[:, 0:W], pap[:, 512 * c : 512 * c + W],
                                         AF.Tanh, bias=pb1[:])
                    nc.vector.tensor_tensor(
                        sq[:, 512 * j : 512 * j + W], vt[:, 0:W], vt[:, 0:W], OP.mult
                    )

        def issue_dz(pend):
            """dz matmuls for a finished SB (deferred one iteration)."""
            k, W, c0k, uts, sq = pend
            dzp = dzpool.tile([128, 1024], F32, tag="dzp", name=f"dzp{k}")
            for half in range(2):
                if W == SBW:
                    utA, utB = uts[half]
                    off, width = 0, W
                else:
                    utA, utB = uts[0]
                    off, width = half * W, W
                for j in range(NCHUNK):
                    p0 = 32 * j
                    utx = utA if j < 2 else utB
                    nc.tensor.matmul(
                        dzp[p0 : p0 + 32, 512 * half : 512 * half + width],
                        w2[:, :],
                        utx[:, 512 * (j % 2) + off : 512 * (j % 2) + off + width],
                        start=True, stop=False,
                        tile_position=(0, p0),
                        skip_group_check=True,
                    )
            return dzp

        def issue_q(pend, dzp):
            """q accumulation, evacuation and output DMA for a finished SB."""
            k, W, c0k, uts, sq = pend
            for sgn, pw in ((0, pwp), (1, pwn)):
                for j in range(NCHUNK):
                    p0 = 32 * j
                    nc.tensor.matmul(
                        dzp[p0 : p0 + 32, 512 * sgn : 512 * sgn + W],
                        pw[:, :],
                        sq[:, 512 * j : 512 * j + W],
                        start=False, stop=(sgn == 1),
                        tile_position=(0, p0),
                        skip_group_check=True,
                    )
            ot = opool.tile([128, 1024], F16, tag="ot", name=f"ot{k}")
            if W == SBW:
                nc.vector.tensor_copy(ot[:, :], dzp[:, :])
                nc.sync.dma_start(O[:, c0k : c0k + 2 * W], ot[:, :])
            else:
                for half in range(2):
                    nc.vector.tensor_copy(
                        ot[:, 512 * half : 512 * half + W],
                        dzp[:, 512 * half : 512 * half + W],
                    )
                    nc.sync.dma_start(
                        O[:, c0k + half * W : c0k + (half + 1) * W],
                        ot[:, 512 * half : 512 * half + W],
                    )

        c0 = 0
        pend = None
        for k, W in enumerate(SBS):
            W2_ = 2 * W
            # halves: (w, cols-per-chunk-strip); tail packs e|o in one strip
            halves = [(0, 512), (1, 512)] if W == SBW else [(0, 2 * W)]
            if k == 0:
                xt = xt0
            elif k == 1:
                # the first big transfer is startup-critical: use two queues
                xt = xpool.tile([128, 2 * SBW], F16, tag="xt", name=f"xt{k}")
                nc.sync.dma_start(xt[:, 0:W], X[:, c0 : c0 + W])
                nc.sync.dma_start(xt[:, W:W2_], X[:, c0 + W : c0 + W2_])
            else:
                xt = xpool.tile([128, 2 * SBW], F16, tag="xt", name=f"xt{k}")
                nc.sync.dma_start(xt[:, :W2_], X[:, c0 : c0 + W2_])

            # pair diffs: df[:, i] = even_i - odd_i
            dft = dfpool.tile([128, SBW], F16)
            nc.gpsimd.tensor_tensor(
                dft[:, :W], xt[:, 0:W], xt[:, W : 2 * W], OP.subtract
            )

            # PE order interleaves the previous SB's dz/q batches between this
            # SB's h/pa groups so the ACT tanh stream never starves:
            #   PE : dz(k-1) hA0 hB0 q(k-1) paA hA1 hB1 paB
            #   ACT: thA0 thB0 thA1 thB1 (continuous)
            #   DVE: evac(k-1), tanhsqA, tanhsqB
            sq = sqpool.tile([128, 2048], F16)
            dzp = issue_dz(pend) if pend is not None else None
            uts = []
            w0, V0 = halves[0]
            utA0 = h_half(xt, w0, V0, 0, k)
            utB0 = h_half(xt, w0, V0, 1, k)
            uts.append((utA0, utB0))
            if pend is not None:
                issue_q(pend, dzp)
            pa_half(0, dft, sq, W, k)
            if len(halves) > 1:
                w1_, V1 = halves[1]
                utA1 = h_half(xt, w1_, V1, 0, k)
                utB1 = h_half(xt, w1_, V1, 1, k)
                uts.append((utA1, utB1))
            pa_half(1, dft, sq, W, k)

            pend = (k, W, c0, uts, sq)
            c0 += W2_
        dzp = issue_dz(pend)
        issue_q(pend, dzp)

    nc.compile()
    return nc


def _prep_weights(t, vW1, vb1, vW2, vb2, pW1, pb1, pW2):
    f32 = np.float32
    t = np.asarray(t, dtype=f32).reshape(-1)[0]
    vW1 = np.asarray(vW1, f32)
    w1rep = np.tile(np.ascontiguousarray(vW1[:32]), (4, 1))            # [128,128]
    biash = (np.asarray(vb1, f32) + t * vW1[32]).reshape(128, 1).astype(f32)
    pW1 = np.asarray(pW1, f32)
    pw1rep = np.tile(pW1, (4, 1))                                      # [128,128]
    pb1c = np.asarray(pb1, f32).reshape(128, 1).copy()
    w2 = np.ascontiguousarray(np.asarray(vW2, f32))                    # [128,32]
    w2col = np.asarray(pW2, f32).reshape(128)
    pwp = np.ascontiguousarray(pW1.T * w2col[:, None])                 # [128,32]
    wcat = np.hstack([w1rep, pw1rep, w2, pwp, -pwp]).astype(np.float16)
    clampc = np.full((128, 1), TSQ_CLAMP, f32)
    bias = np.hstack([biash, pb1c, clampc]).astype(f32)
    # constant part of q: c0[d] = sum_h pW1[d,h]*pW2[h], in fp16 weight precision
    c0base = wcat[:, 288:320].astype(f32).sum(axis=0)                  # [32]
    return {"wcat": np.ascontiguousarray(wcat), "bias": np.ascontiguousarray(bias),
            "_c0base": c0base}


def _pack_core(zc):
    """[25000, 32] f32 -> [128, 6272] fp16: partition 32j+d = dim d of chunk j;
    cols per superblock: [W even rows | W odd rows]."""
    zp = np.zeros((NCHUNK, HALFP, 2, 32), dtype=np.float16)
    zp[:, :PAIRS_PC] = zc.reshape(NCHUNK, PAIRS_PC, 2, 32)
    out = np.empty((128, XC), dtype=np.float16)
    c0 = 0
    for (p0, W), _ in zip(SB_PAIRS, SBS):
        blk = zp[:, p0 : p0 + W]                            # [4, W, 2, 32]
        out[:, c0 : c0 + W] = blk[:, :, 0].transpose(0, 2, 1).reshape(128, W)
        out[:, c0 + W : c0 + 2 * W] = blk[:, :, 1].transpose(0, 2, 1).reshape(128, W)
        c0 += 2 * W
    return out


def _unpack_core(oc):
    """[128, 6272] fp16 -> even [4,HALFP,32], odd [4,HALFP,32] (f32)."""
    ev = np.empty((NCHUNK, HALFP, 32), dtype=np.float32)
    od = np.empty((NCHUNK, HALFP, 32), dtype=np.float32)
    c0 = 0
    for p0, W in SB_PAIRS:
        ev[:, p0 : p0 + W] = (
            oc[:, c0 : c0 + W].astype(np.float32).reshape(NCHUNK, 32, W).transpose(0, 2, 1)
        )
        od[:, p0 : p0 + W] = (
            oc[:, c0 + W : c0 + 2 * W].astype(np.float32).reshape(NCHUNK, 32, W).transpose(0, 2, 1)
        )
        c0 += 2 * W
    return ev, od


def _host_triple(t, z3, vW1, vb1, vW2, vb2, pW1, pb1, pW2):
    """Exact float64 computation of the 3 leftover rows: dz_dt + triple forces."""
    f8 = np.float64
    z3 = z3.astype(f8)
    vW1 = np.asarray(vW1, f8)
    t = float(np.asarray(t).reshape(-1)[0])
    h3 = np.tanh(z3 @ vW1[:32] + t * vW1[32] + np.asarray(vb1, f8))
    dz3 = h3 @ np.asarray(vW2, f8) + np.asarray(vb2, f8)

    pW1 = np.asarray(pW1, f8)
    w2 = np.asarray(pW2, f8).reshape(128)
    d9 = (z3[:, None, :] - z3[None, :, :]).reshape(9, 32)
    u9 = np.tanh(d9 @ pW1 + np.asarray(pb1, f8))
    s9 = (1.0 - u9 * u9) * w2[None, :]
    g9 = s9 @ pW1.T
    f9 = (-g9).reshape(3, 3, 32)
    f9 = f9 * (1.0 - np.eye(3)[:, :, None])
    force3 = f9.sum(axis=1) * 2.0
    return (dz3 + force3).astype(np.float32)


def kernel(t, z, perm, vW1, vb1, vW2, vb2, pW1, pb1, pW2, pb2):
    from concourse.bass_utils import run_bass_kernel_spmd

    global LAST_RESULTS
    use_tanhsq = bool(np.all(np.asarray(pb1) == 0))
    key = ("nc", use_tanhsq)
    if key not in _CACHE:
        _CACHE[key] = build_program(use_tanhsq)
    nc = _CACHE[key]

    z = np.asarray(z, np.float32)
    perm = np.asarray(perm)
    weights = _prep_weights(t, vW1, vb1, vW2, vb2, pW1, pb1, pW2)
    c0base = weights.pop("_c0base")

    zg = z[perm[:P2]]                       # [200000, 32] gathered pair rows
    in_maps = []
    for c in range(NCORES):
        im = {"x": _pack_core(zg[c * RPC : (c + 1) * RPC])}
        im.update(weights)
        in_maps.append(im)

    trace = bool(int(os.environ.get("KERNEL_TRACE", "0")))
    res = run_bass_kernel_spmd(nc, in_maps, list(range(NCORES)), trace=trace)
    LAST_RESULTS = res

    vb2f = np.asarray(vb2, np.float32)
    add_e = (vb2f - c0base)[None, :]
    add_o = (vb2f + c0base)[None, :]
    out = np.empty((B, 32), dtype=np.float32)
    og = np.empty((RPC * NCORES, 32), dtype=np.float32)
    for c in range(NCORES):
        ev, od = _unpack_core(res.results[c]["out"])
        blk = np.empty((NCHUNK, PAIRS_PC, 2, 32), dtype=np.float32)
        blk[:, :, 0] = ev[:, :PAIRS_PC] + add_e
        blk[:, :, 1] = od[:, :PAIRS_PC] + add_o
        og[c * RPC : (c + 1) * RPC] = blk.reshape(RPC, 32)
    out[perm[:P2]] = og
    out[perm[P2:]] = _host_triple(t, z[perm[P2:]], vW1, vb1, vW2, vb2, pW1, pb1, pW2)
    return out
